# revision 16
# baseline (speedup 1.0000x reference)
"""Distributed Trainium2 kernel for nn_AdaptivePooling (sliding-window
mean/max/logvar pooling + linear projection).

Reference computation (B=64, D=256, T=4096, kernel=16, stride=8, N=511):
    win[b,d,n,:] = x[b, d, 8n : 8n+16]
    pooled = w0*mean(win) + w1*max(win) + w2*log(clip(var_unbiased(win)))
    out[b,e,n] = sum_d proj_w[e,d] * pooled[b,d,n] + proj_b[e]
with [w0,w1,w2] = softmax(pool_weights).

Strategy: data-parallel over batch across 8 NeuronCores (8 batches/core).
Variance must be computed in centered (two-pass) form: the input contains
near-constant windows (var ~ 1e-6) where the one-pass ssq - sum^2/k form
cancels catastrophically and log() amplifies the error.

Per batch one [128, 2, 4096] f32 tile (d = h*128 + partition):
  - sum8[c] chunk sums: 8 accumulated identity-matmuls on TensorE
    (stride-8 rhs slices), per half -> PSUM -> SBUF
  - DEV = x - sum8[c]/8 broadcast (VectorE), SQD = DEV^2 (ScalarE, bf16,
    in place), M2c8 = segmented reduce_sum on VectorE (contiguous)
  - max8 via a bf16 fold tree on VectorE
  - window stats (chunks n, n+1), Chan-combined (all terms >= 0):
      q = M2c8[n] + M2c8[n+1] + (sum8[n]-sum8[n+1])^2 / 16
      log(var_unbiased) = Ln(q * (1/15)), q clipped to [15e-6, 15e6]
    small elementwise ops on GpSimd
  - projection folds softmax weights into host-prefolded bf16 weights:
      Wcat = [w0/16*W | w1*W | w2*W],  rhs = [sum16; max16; ln(q/15)]
"""

import numpy as np

B, D, T = 64, 256, 4096
KER, STR = 16, 8
N = (T - KER) // STR + 1  # 511
C = T // STR  # 512 chunks
N_CORES = 8
BL = B // N_CORES  # 8 batches per core

_CACHE: dict = {}


def _build(reps=1, pe_sum8=False, pe_m2=False, m2_tree=False, dev_split=False,
           x_bufs=2):
    from concourse import bacc, mybir, tile

    F32 = mybir.dt.float32
    BF16 = mybir.dt.bfloat16
    ALU = mybir.AluOpType
    ACT = mybir.ActivationFunctionType
    AX = mybir.AxisListType.X

    nc = bacc.Bacc("TRN2", target_bir_lowering=False, debug=False,
                   num_devices=N_CORES)
    x_ext = nc.dram_tensor("x", [BL, D, T], F32, kind="ExternalInput").ap()
    wt_ext = nc.dram_tensor("wt", [128, 6, 256], BF16, kind="ExternalInput").ap()
    beff_ext = nc.dram_tensor("beff", [128, 2], F32, kind="ExternalInput").ap()
    eyef_ext = nc.dram_tensor("eyef", [128, 128], F32, kind="ExternalInput").ap()
    eyeb_ext = nc.dram_tensor("eyeb", [128, 128], BF16, kind="ExternalInput").ap()
    out_ext = nc.dram_tensor("out", [BL, D, N], F32, kind="ExternalOutput").ap()

    with tile.TileContext(nc) as tc:
        with (
            tc.tile_pool(name="wpool", bufs=1) as wpool,
            tc.tile_pool(name="xpool", bufs=x_bufs) as xpool,
            tc.tile_pool(name="devp", bufs=2) as devp,
            tc.tile_pool(name="m1p", bufs=2) as m1p,
            tc.tile_pool(name="m2p", bufs=2) as m2p,
            tc.tile_pool(name="r8", bufs=2) as r8,
            tc.tile_pool(name="small", bufs=2) as small,
            tc.tile_pool(name="stats", bufs=2) as stpool,
            tc.tile_pool(name="opool", bufs=3) as opool,
            tc.tile_pool(name="ps_s8", bufs=2, space="PSUM") as ps_s8p,
            tc.tile_pool(name="ps_m2", bufs=2, space="PSUM") as ps_m2p,
            tc.tile_pool(name="ps_o", bufs=4, space="PSUM") as ps_op,
        ):
            wt = wpool.tile([128, 6, 256], BF16)
            nc.sync.dma_start(wt[:], wt_ext[:])
            beff = wpool.tile([128, 2], F32)
            nc.sync.dma_start(beff[:], beff_ext[:])
            eyef = wpool.tile([128, 128], F32)
            nc.sync.dma_start(eyef[:], eyef_ext[:])
            eyeb = wpool.tile([128, 128], BF16)
            nc.sync.dma_start(eyeb[:], eyeb_ext[:])

            rep_ctx = tc.For_i(0, reps, 1) if reps > 1 else None
            if rep_ctx is not None:
                rep_ctx.__enter__()
            for b in range(BL):
                X = xpool.tile([128, 2, T], F32, tag="x")
                nc.sync.dma_start(
                    X[:], x_ext[b].rearrange("(h p) t -> p h t", p=128))
                X4 = X[:].rearrange("p h (c k) -> p h c k", k=8)

                # --- chunk sums ---
                sum8 = r8.tile([128, 2, C], F32, tag="sum8")
                if pe_sum8:
                    for h in range(2):
                        ps = ps_s8p.tile([128, C], F32, tag="ps_s8")
                        for j in range(8):
                            nc.tensor.matmul(ps[:], eyef[:], X4[:, h, :, j],
                                             start=(j == 0), stop=(j == 7))
                        nc.scalar.copy(sum8[:, h, :], ps[:])
                else:
                    nc.vector.reduce_sum(sum8[:], X4, axis=AX)

                # --- centered deviations, squared ---
                DEV = devp.tile([128, 2, T], BF16, tag="dev")
                DEV4 = DEV[:].rearrange("p h (c k) -> p h c k", k=8)
                if dev_split:
                    # half on VectorE (stt), half on GpSimd (ts + add)
                    s0b = sum8[:, 0].rearrange("p (c o) -> p c o", o=1) \
                                    .broadcast_to([128, C, 8])
                    nc.vector.scalar_tensor_tensor(
                        DEV4[:, 0], s0b, -0.125, X4[:, 0],
                        op0=ALU.mult, op1=ALU.add)
                    nm8 = small.tile([128, C], F32, tag="nm8")
                    nc.gpsimd.tensor_scalar(
                        nm8[:], sum8[:, 1, :], -0.125, None, op0=ALU.mult)
                    nm8b = nm8[:].rearrange("p (c o) -> p c o", o=1) \
                                 .broadcast_to([128, C, 8])
                    nc.gpsimd.tensor_tensor(
                        DEV4[:, 1], X4[:, 1], nm8b, op=ALU.add)
                else:
                    sum8b = sum8[:].rearrange("p h (c o) -> p h c o", o=1) \
                                   .broadcast_to([128, 2, C, 8])
                    nc.vector.scalar_tensor_tensor(
                        DEV4, sum8b, -0.125, X4, op0=ALU.mult, op1=ALU.add)
                nc.scalar.activation(DEV[:], DEV[:], ACT.Square)  # in place
                m2c8 = r8.tile([128, 2, C], F32, tag="m2c8")
                if pe_m2:
                    for h in range(2):
                        ps = ps_m2p.tile([128, C], F32, tag="ps_m2")
                        for j in range(8):
                            nc.tensor.matmul(ps[:], eyeb[:], DEV4[:, h, :, j],
                                             start=(j == 0), stop=(j == 7))
                        nc.scalar.copy(m2c8[:, h, :], ps[:])
                elif m2_tree:
                    # bf16 fold tree (2x perf modes); squares of deviations
                    # keep full relative precision so bf16 partials are safe
                    Q1 = m1p.tile([128, 2, C, 4], BF16, tag="q1")
                    nc.vector.tensor_tensor(
                        Q1[:], DEV4[:, :, :, 0:4], DEV4[:, :, :, 4:8],
                        op=ALU.add)
                    Q2 = m2p.tile([128, 2, C, 2], BF16, tag="q2")
                    nc.vector.tensor_tensor(
                        Q2[:], Q1[:, :, :, 0:2], Q1[:, :, :, 2:4], op=ALU.add)
                    nc.vector.tensor_tensor(
                        m2c8[:], Q2[:, :, :, 0], Q2[:, :, :, 1], op=ALU.add)
                else:
                    nc.vector.reduce_sum(m2c8[:], DEV4, axis=AX)

                # --- max8 fold tree on VectorE ---
                M1 = m1p.tile([128, 2, C, 4], BF16, tag="m1")
                nc.vector.tensor_tensor(
                    M1[:], X4[:, :, :, 0:4], X4[:, :, :, 4:8], op=ALU.max)
                M2x = m2p.tile([128, 2, C, 2], BF16, tag="m2x")
                nc.vector.tensor_tensor(
                    M2x[:], M1[:, :, :, 0:2], M1[:, :, :, 2:4], op=ALU.max)
                max8 = r8.tile([128, 2, C], BF16, tag="max8")
                nc.vector.tensor_tensor(
                    max8[:], M2x[:, :, :, 0], M2x[:, :, :, 1], op=ALU.max)

                # --- window (16) stats ---
                st = stpool.tile([128, 2, 3, N], BF16, tag="st")
                nc.gpsimd.tensor_tensor(
                    st[:, :, 0, :], sum8[:, :, 0:N], sum8[:, :, 1:C],
                    op=ALU.add)
                nc.vector.tensor_tensor(
                    st[:, :, 1, :], max8[:, :, 0:N], max8[:, :, 1:C],
                    op=ALU.max)
                m2c16 = small.tile([128, 2, N], F32, tag="m2c16")
                nc.gpsimd.tensor_tensor(
                    m2c16[:], m2c8[:, :, 0:N], m2c8[:, :, 1:C], op=ALU.add)
                d8 = small.tile([128, 2, N], F32, tag="d8")
                nc.gpsimd.tensor_tensor(
                    d8[:], sum8[:, :, 0:N], sum8[:, :, 1:C], op=ALU.subtract)
                nc.gpsimd.tensor_tensor(d8[:], d8[:], d8[:], op=ALU.mult)
                nc.gpsimd.tensor_scalar(
                    d8[:], d8[:], 1.0 / 16.0, None, op0=ALU.mult)
                nc.gpsimd.tensor_tensor(m2c16[:], m2c16[:], d8[:], op=ALU.add)
                nc.gpsimd.tensor_scalar(
                    m2c16[:], m2c16[:], 15e-6, 15e6, op0=ALU.max, op1=ALU.min)
                nc.scalar.activation(st[:, :, 2, :], m2c16[:], ACT.Ln,
                                     scale=1.0 / 15.0)

                # --- projection ---
                for eh in range(2):
                    ps = ps_op.tile([128, N], F32, tag="ps_o")
                    k = 0
                    for s in range(3):
                        for h in range(2):
                            nc.tensor.matmul(
                                ps[:],
                                wt[:, s * 2 + h, eh * 128:(eh + 1) * 128],
                                st[:, h, s, :],
                                start=(k == 0), stop=(k == 5))
                            k += 1
                    ob = opool.tile([128, N], F32, tag="ob")
                    nc.scalar.activation(ob[:], ps[:], ACT.Identity,
                                         bias=beff[:, eh:eh + 1], scale=1.0)
                    nc.sync.dma_start(out_ext[b, eh * 128:(eh + 1) * 128, :], ob[:])

            if rep_ctx is not None:
                rep_ctx.__exit__(None, None, None)

    nc.compile()
    return nc


def _get_nc():
    if "nc" not in _CACHE:
        _CACHE["nc"] = _build()
    return _CACHE["nc"]


def _prep_host(pool_weights, proj_w, proj_b):
    from concourse import mybir
    BF16_NP = mybir.dt.np(mybir.dt.bfloat16)

    pw = np.asarray(pool_weights, np.float32)
    e = np.exp(pw - pw.max())
    w = (e / e.sum()).astype(np.float32)

    W = np.asarray(proj_w, np.float32)  # [E, D]
    Wcat = np.concatenate(
        [(w[0] / 16.0) * W, w[1] * W, w[2] * W], axis=1)  # [256, 768]
    lhsT = np.ascontiguousarray(Wcat.T)  # [768, 256]
    wt_host = np.ascontiguousarray(
        lhsT.reshape(6, 128, 256).transpose(1, 0, 2)).astype(BF16_NP)
    beff_host = np.ascontiguousarray(
        np.asarray(proj_b, np.float32).reshape(2, 128).T)
    eyef = np.eye(128, dtype=np.float32)
    eyeb = np.eye(128, dtype=np.float32).astype(BF16_NP)
    return wt_host, beff_host, eyef, eyeb, BF16_NP


def _get_runner():
    """Cached jitted SPMD runner (avoids re-tracing the PJRT wrapper on
    every kernel() call).  Mirrors bass2jax.run_bass_via_pjrt."""
    if "runner" in _CACHE:
        return _CACHE["runner"]

    import jax
    from concourse import mybir
    from concourse.bass2jax import (
        _bass_exec_p, install_neuronx_cc_hook, partition_id_tensor)
    from jax.sharding import Mesh, PartitionSpec
    from jax.experimental.shard_map import shard_map

    nc = _get_nc()
    install_neuronx_cc_hook()

    partition_name = (nc.partition_id_tensor.name
                      if nc.partition_id_tensor else None)
    in_names, out_names, out_avals, zero_shapes = [], [], [], []
    for alloc in nc.m.functions[0].allocations:
        if not isinstance(alloc, mybir.MemoryLocationSet):
            continue
        name = alloc.memorylocations[0].name
        if alloc.kind == "ExternalInput":
            if name != partition_name:
                in_names.append(name)
        elif alloc.kind == "ExternalOutput":
            out_names.append(name)
            shape = tuple(alloc.tensor_shape)
            dtype = mybir.dt.np(alloc.dtype)
            out_avals.append(jax.core.ShapedArray(shape, dtype))
            zero_shapes.append((shape, dtype))
    n_params = len(in_names)
    n_outs = len(out_avals)
    all_in = in_names + out_names + ([partition_name] if partition_name else [])

    def _body(*args):
        operands = list(args)
        if partition_name is not None:
            operands.append(partition_id_tensor())
        outs = _bass_exec_p.bind(
            *operands, out_avals=tuple(out_avals), in_names=tuple(all_in),
            out_names=tuple(out_names), lowering_input_output_aliases=(),
            sim_require_finite=True, sim_require_nnan=True, nc=nc)
        return tuple(outs)

    devices = jax.devices()[:N_CORES]
    mesh = Mesh(np.asarray(devices), ("core",))
    in_specs = (PartitionSpec("core"),) * (n_params + n_outs)
    out_specs = (PartitionSpec("core"),) * n_outs
    donate = tuple(range(n_params, n_params + n_outs))
    sharded = jax.jit(
        shard_map(_body, mesh=mesh, in_specs=in_specs, out_specs=out_specs,
                  check_rep=False),
        donate_argnums=donate, keep_unused=True)
    sharding = jax.sharding.NamedSharding(mesh, PartitionSpec("core"))

    def run(in_maps):
        concat_in = [
            np.concatenate(
                [np.asarray(in_maps[c][nm]) for c in range(N_CORES)], axis=0)
            for nm in in_names
        ]
        dev_in = [jax.device_put(a, sharding) for a in concat_in]
        zs = [
            jax.device_put(
                np.zeros((N_CORES * s[0], *s[1:]), dt), sharding)
            for (s, dt) in zero_shapes
        ]
        outs = sharded(*dev_in, *zs)
        return {
            nm: np.asarray(outs[i]).reshape(N_CORES, *out_avals[i].shape)
            for i, nm in enumerate(out_names)
        }

    _CACHE["runner"] = run
    return run


def kernel(x, pool_weights, proj_w, proj_b):
    wt_host, beff_host, eyef, eyeb, _ = _prep_host(pool_weights, proj_w, proj_b)
    x_f = np.ascontiguousarray(np.asarray(x, np.float32))

    in_maps = [
        {"x": x_f[i * BL:(i + 1) * BL], "wt": wt_host, "beff": beff_host,
         "eyef": eyef, "eyeb": eyeb}
        for i in range(N_CORES)
    ]
    res = _get_runner()(in_maps)
    out = res["out"].reshape(B, D, N)
    return np.ascontiguousarray(out.astype(np.float32))
